# revision 18
# baseline (speedup 1.0000x reference)
"""AFM (Attentional Factorization Machine) Trainium2 kernel, 8-core data-parallel.

Model (per batch row b, F=20 fields, D=64 emb, A=32 att dims, P=190 pairs):
  e[f]   = emb_table[features[f]] * fv[f]
  ele[p] = e[i_p] * e[j_p]                       (elementwise, 190 upper-tri pairs)
  z[p]   = W1 @ ele[p] + b1 ; h = relu(z)
  l[p]   = w2 . h[p] ; att = softmax_p(l)
  pred   = sum_p att[p] * (ele[p] . pred_w) + sum_f bias[features[f]]*fv[f] + gb

Mapping (per core, 1024 batch rows = 8 tiles of 128, processed as 4 "ddt" of 2 tiles):
  - indirect-DMA gather of padded bf16 table rows -> eA [128b, 20f, 80] (col64 = bias)
  - fv pre-fold on eA (DVE), fb row-bias reduce
  - PE transposes eA fields -> eB [128 = 64d x 2 tiles, 20f*128b] bf16
  - pairwise products by gap shift (DVE TT bf16 2x) -> ele [128, 190*128]
  - mm1: Wpack [128,66] = blockdiag(|w2|*W1^T x2) + pred_w cols -> z+s in PSUM [66,512]
  - drain: relu(z + |w2|*b1) via per-partition (bias,max-floor) select; s passes through
  - mm2: shifted selector stationaries -> l/s of 16 chunks land across 128 PSUM partitions
  - shuffle/exp/mult + selector matmuls -> softmax num/den per batch row; combine + fb + gb
"""

import numpy as np
import ml_dtypes
from contextlib import ExitStack

import concourse.bass as bass
import concourse.mybir as mybir
import concourse.tile as tile
from concourse import bacc
from concourse.bass import IndirectOffsetOnAxis
from concourse.bass_utils import run_bass_kernel_spmd
from concourse.masks import make_identity

BF16 = mybir.dt.bfloat16
F32 = mybir.dt.float32
I32 = mybir.dt.int32

V = 100000
ROW = 80          # 64 emb + 1 bias + 15 pad (bf16) = 160B gather per row
Fdim = 20
D = 64
A = 32
P = 190
NCORES = 8
NT = 8            # 128-row batch tiles per core
NDDT = 4          # tile pairs
FILL_PAIRS = (64, 64, 62)   # pairs per mm2 fill (16 chunks of <=512 cols each)

AluOp = mybir.AluOpType
ActFn = mybir.ActivationFunctionType

_CACHE = {}


def _build(gb: float):
    nc = bacc.Bacc("TRN2", target_bir_lowering=False)

    table = nc.declare_dram_parameter("table", [V, ROW], BF16, isOutput=False)
    idx = nc.declare_dram_parameter("idx", [NT, 128, Fdim], I32, isOutput=False)
    fv = nc.declare_dram_parameter("fv", [NT, 128, Fdim], BF16, isOutput=False)
    wpack = nc.declare_dram_parameter("wpack", [128, 66], BF16, isOutput=False)
    spack = nc.declare_dram_parameter("spack", [66, 128], BF16, isOutput=False)
    selmat = nc.declare_dram_parameter("selmat", [128, 4], BF16, isOutput=False)
    b1sel = nc.declare_dram_parameter("b1sel", [66, 2], F32, isOutput=False)
    padconst = nc.declare_dram_parameter("padconst", [128, 4], F32, isOutput=False)
    out = nc.declare_dram_parameter("out", [NT, 128], F32, isOutput=True)

    shuf_mask = [(i - 2 if i % 8 in (2, 3) else i) for i in range(32)]

    with tile.TileContext(nc) as tc, ExitStack() as ctx:
        const = ctx.enter_context(tc.tile_pool(name="const", bufs=1))
        inpool = ctx.enter_context(tc.tile_pool(name="inp", bufs=8))
        eApool = ctx.enter_context(tc.tile_pool(name="eA", bufs=8))
        eBpool = ctx.enter_context(tc.tile_pool(name="eB", bufs=2))
        elepool = ctx.enter_context(tc.tile_pool(name="ele", bufs=2))
        zrpool = ctx.enter_context(tc.tile_pool(name="zr", bufs=4))
        lspool = ctx.enter_context(tc.tile_pool(name="ls", bufs=3))
        smpool = ctx.enter_context(tc.tile_pool(name="sm", bufs=4))
        petp = ctx.enter_context(tc.tile_pool(name="peT", bufs=1, space="PSUM"))
        zpsum = ctx.enter_context(tc.tile_pool(name="zps", bufs=1, space="PSUM"))
        lpsum = ctx.enter_context(tc.tile_pool(name="lps", bufs=1, space="PSUM"))
        dpsum = ctx.enter_context(tc.tile_pool(name="dps", bufs=1, space="PSUM"))

        ident_bf = const.tile([128, 128], BF16)
        make_identity(nc, ident_bf[:])
        ident_f = const.tile([128, 128], F32)
        make_identity(nc, ident_f[:])
        w_t = const.tile([128, 66], BF16)
        nc.sync.dma_start(w_t[:], wpack[:, :])
        s_t = const.tile([66, 128], BF16)
        nc.sync.dma_start(s_t[:], spack[:, :])
        sel_t = const.tile([128, 4], BF16)
        nc.sync.dma_start(sel_t[:], selmat[:, :])
        b1_t = const.tile([66, 2], F32)
        nc.sync.dma_start(b1_t[:], b1sel[:, :])
        padc = const.tile([128, 4], F32)
        nc.sync.dma_start(padc[:], padconst[:, :])
        fb_sb = const.tile([128, NT], F32)

        for d in range(NDDT):
            eAs = []
            for par in (0, 1):
                t = 2 * d + par
                it = inpool.tile([128, Fdim], I32, tag="idx")
                nc.sync.dma_start(it[:], idx[t, :, :])
                it2 = inpool.tile([128, Fdim], I32, tag="idx2")
                nc.gpsimd.tensor_copy(it2[:], it[:])
                ft = inpool.tile([128, Fdim], BF16, tag="fv")
                nc.sync.dma_start(ft[:], fv[t, :, :])
                ftc = inpool.tile([128, Fdim], BF16, tag="fvc")
                nc.vector.tensor_copy(ftc[:], ft[:])
                eA = eApool.tile([128, Fdim, ROW], BF16)
                for f in range(Fdim):
                    nc.gpsimd.indirect_dma_start(
                        out=eA[:, f, :],
                        out_offset=None,
                        in_=table[:, :],
                        in_offset=IndirectOffsetOnAxis(ap=it2[:, f:f + 1], axis=0),
                    )
                # 2D join op: absorbs gather+copy waits so the 3D prefold
                # below rides DVE program order (3D TT has 1 wait slot)
                jn = inpool.tile([128, 8], BF16, tag="join")
                nc.vector.tensor_copy(
                    jn[:], eA[:, 0:1, 0:8].rearrange("p a b -> p (a b)"))
                # fv pre-fold (scales emb cols and the bias col together)
                nc.vector.tensor_tensor(
                    out=eA[:], in0=eA[:],
                    in1=ftc[:].to_broadcast([128, Fdim, ROW]),
                    op=AluOp.mult,
                )
                # fb[b] = sum_f bias*fv  (bias col 64, already fv-scaled)
                nc.vector.tensor_reduce(
                    out=fb_sb[:, t:t + 1],
                    in_=eA[:, :, 64:65].rearrange("p f one -> p (f one)"),
                    axis=mybir.AxisListType.X,
                    op=AluOp.add,
                )
                eAs.append(eA)

            # transpose e to [d-on-partitions]: eB[64*par + d, f*128 + b]
            eB = eBpool.tile([128, Fdim * 128], BF16)
            for fg in range(5):  # 4 fields per PSUM tile
                pe = petp.tile([128, 4 * 128], BF16)
                for ff in range(4):
                    f = fg * 4 + ff
                    for par in (0, 1):
                        nc.tensor.matmul(
                            pe[64 * par:64 * par + 64, ff * 128:(ff + 1) * 128],
                            lhsT=eAs[par][:, f, 0:64],
                            rhs=ident_bf[:],
                            is_transpose=True,
                        )
                nc.scalar.copy(eB[:, fg * 512:(fg + 1) * 512], pe[:])

            # pairwise products, gap-major pair order
            ele = elepool.tile([128, P * 128], BF16)
            col = 0
            for g in range(1, Fdim):
                n = Fdim - g
                nc.vector.tensor_tensor(
                    out=ele[:, col:col + n * 128],
                    in0=eB[:, 0:n * 128],
                    in1=eB[:, g * 128:(g + n) * 128],
                    op=AluOp.mult,
                )
                col += n * 128

            # attention MLP + per-pair scores, 16-chunk fills processed as
            # 4 groups of 4 chunks: mm1 x4 same-stationary into a 4-bank
            # PSUM batch, one wide drain, mm2 x4 same-stationary.
            # chunk k lands at slot partitions 32*(k%4) + 8*(k//4) + j
            # (den/num sums are slot-order independent).
            pden = dpsum.tile([64, 512], F32, tag="dps")
            colbase = 0
            for f3, npairs in enumerate(FILL_PAIRS):
                pls = lpsum.tile([128, 512], F32)
                ncols = npairs * 128
                for k4 in range(4):
                    gw = min(2048, ncols - k4 * 2048)
                    pz = zpsum.tile([66, 2048], F32)
                    for kk in range(4):
                        c0 = k4 * 2048 + kk * 512
                        w = min(512, ncols - c0)
                        if w <= 0:
                            break
                        nc.tensor.matmul(
                            pz[:, kk * 512:kk * 512 + w], lhsT=w_t[:, 0:66],
                            rhs=ele[:, colbase + c0: colbase + c0 + w],
                            start=True, stop=True,
                        )
                    zr = zrpool.tile([66, 2048], BF16)
                    if k4 % 3 == 2:
                        nc.scalar.activation(zr[0:64, 0:gw], pz[0:64, 0:gw],
                                             ActFn.Relu, bias=b1_t[0:64, 0:1])
                        nc.scalar.copy(zr[64:66, 0:gw], pz[64:66, 0:gw])
                    else:
                        nc.vector.tensor_scalar(
                            out=zr[:, 0:gw], in0=pz[:, 0:gw],
                            scalar1=b1_t[:, 0:1], scalar2=b1_t[:, 1:2],
                            op0=AluOp.add, op1=AluOp.max,
                        )
                    for kk in range(4):
                        w = min(512, ncols - (k4 * 2048 + kk * 512))
                        if w <= 0:
                            break
                        base = 32 * kk
                        nc.tensor.matmul(
                            pls[base:base + 32, 0:w],
                            lhsT=s_t[:, 32 * k4:32 * (k4 + 1)],
                            rhs=zr[:, kk * 512:kk * 512 + w],
                            start=(k4 == 0), stop=(k4 == 3),
                            skip_group_check=True,
                            tile_position=(0, base),
                        )
                colbase += ncols

                ls = lspool.tile([128, 512], BF16, tag="ls")
                nc.vector.tensor_copy(ls[:], pls[:])
                if npairs < 64:
                    # phantom pair slots (partitions 120-127, cols 256:512):
                    # keep 96-119 (mult 1, add 0), force 120-127 to -3e38
                    pad_ap = ls[96:128, 256:512]
                    nc.vector.tensor_scalar(out=pad_ap, in0=pad_ap,
                                            scalar1=padc[96:128, 0:1],
                                            scalar2=padc[96:128, 1:2],
                                            op0=AluOp.mult, op1=AluOp.add)
                t1 = lspool.tile([128, 512], BF16, tag="t1")
                nc.vector.stream_shuffle(t1[:], ls[:], shuf_mask)
                e1 = lspool.tile([128, 512], BF16, tag="e1")
                nc.scalar.activation(e1[:], t1[:], ActFn.Exp)
                pr = lspool.tile([128, 512], BF16, tag="pr")
                nc.vector.tensor_tensor(pr[:], e1[:], ls[:], op=AluOp.mult)
                nc.tensor.matmul(pden[0:2, :], lhsT=sel_t[:, 0:2], rhs=e1[:],
                                 start=(f3 == 0), stop=(f3 == 2),
                                 skip_group_check=True)
                nc.tensor.matmul(pden[32:34, :], lhsT=sel_t[:, 2:4], rhs=pr[:],
                                 start=(f3 == 0), stop=(f3 == 2),
                                 skip_group_check=True)

            # softmax combine: den/num [2, (4 p4, 128 b)] -> pred [2, 128]
            den2 = smpool.tile([2, 128], F32, tag="den")
            nc.vector.tensor_reduce(
                out=den2[:], in_=pden[0:2, :].rearrange("p (f b) -> p b f", f=4),
                axis=mybir.AxisListType.X, op=AluOp.add)
            num2 = smpool.tile([2, 128], F32, tag="num")
            nc.vector.tensor_reduce(
                out=num2[:], in_=pden[32:34, :].rearrange("p (f b) -> p b f", f=4),
                axis=mybir.AxisListType.X, op=AluOp.add)
            rcp = smpool.tile([2, 128], F32, tag="rcp")
            nc.vector.reciprocal(rcp[:], den2[:])
            pfb = dpsum.tile([2, 128], F32, tag="fbT")
            nc.tensor.matmul(pfb[:], lhsT=fb_sb[:, 2 * d:2 * d + 2],
                             rhs=ident_f[:], is_transpose=True)
            pred = smpool.tile([2, 128], F32, tag="pred")
            nc.vector.tensor_tensor(pred[:], num2[:], rcp[:], op=AluOp.mult)
            nc.vector.tensor_tensor(pred[:], pred[:], pfb[:], op=AluOp.add)
            nc.vector.tensor_scalar(out=pred[:], in0=pred[:], scalar1=float(gb),
                                    scalar2=None, op0=AluOp.add)
            nc.sync.dma_start(out[2 * d:2 * d + 2, :], pred[:])

    nc.compile()
    return nc


def _prep_consts(inputs):
    bf16 = ml_dtypes.bfloat16
    emb = np.asarray(inputs["emb_table"], np.float32)
    bias = np.asarray(inputs["bias_table"], np.float32).reshape(-1)
    w1 = np.asarray(inputs["att_w1"], np.float32)
    b1 = np.asarray(inputs["att_b1"], np.float32)
    w2 = np.asarray(inputs["att_w2"], np.float32)
    pw = np.asarray(inputs["pred_w"], np.float32)
    gb = float(np.asarray(inputs["global_bias"]).reshape(-1)[0])

    table = np.zeros((V, ROW), bf16)
    table[:, :D] = emb.astype(bf16)
    table[:, D] = bias.astype(bf16)

    w2a = np.abs(w2)
    sg = np.sign(w2).astype(np.float32)
    w1eff = w1 * w2a[:, None]          # [A, D]
    b1eff = b1 * w2a

    wpack = np.zeros((128, 66), np.float32)
    wpack[0:D, 0:A] = w1eff.T
    wpack[D:2 * D, A:2 * A] = w1eff.T
    wpack[0:D, 64] = pw
    wpack[D:2 * D, 65] = pw

    b1sel = np.zeros((66, 2), np.float32)
    b1sel[0:A, 0] = b1eff
    b1sel[A:2 * A, 0] = b1eff
    b1sel[64:66, 1] = -3.0e38

    spack = np.zeros((66, 128), np.float32)
    for q in range(4):
        c = 32 * q + 8 * q
        spack[0:A, c + 0] = sg
        spack[A:2 * A, c + 1] = sg
        spack[64, c + 2] = 1.0
        spack[65, c + 3] = 1.0

    selmat = np.zeros((128, 4), np.float32)
    for k in range(16):
        for g2 in range(2):
            selmat[8 * k + g2, g2] = 1.0
            selmat[8 * k + 2 + g2, 2 + g2] = 1.0

    padconst = np.zeros((128, 4), np.float32)
    padconst[:, 0] = 1.0
    padconst[120:128, 0] = 0.0
    padconst[120:128, 1] = -3.0e38

    return dict(
        table=table, wpack=wpack.astype(bf16), spack=spack.astype(bf16),
        selmat=selmat.astype(bf16), b1sel=b1sel, padconst=padconst,
    ), gb


TRACE = [False]


def kernel(**inputs):
    bf16 = ml_dtypes.bfloat16
    feats = np.asarray(inputs["features"]).astype(np.int32)
    fvals = np.asarray(inputs["feature_values"], np.float32)
    B = feats.shape[0]
    assert B == NCORES * NT * 128

    consts, gb = _prep_consts(inputs)
    idx_all = feats.reshape(NCORES, NT, 128, Fdim)
    fv_all = fvals.astype(bf16).reshape(NCORES, NT, 128, Fdim)

    key = "nc"
    if key not in _CACHE or _CACHE.get("gb") != gb:
        _CACHE[key] = _build(gb)
        _CACHE["gb"] = gb
    nc = _CACHE[key]

    in_maps = [
        {**consts, "idx": idx_all[i], "fv": fv_all[i]} for i in range(NCORES)
    ]
    res = run_bass_kernel_spmd(nc, in_maps, list(range(NCORES)),
                               trace=TRACE[0])
    if TRACE[0]:
        _CACHE["last_res"] = res
    outs = [np.asarray(res.results[i]["out"]).reshape(-1) for i in range(NCORES)]
    return np.concatenate(outs).astype(np.float32)


if __name__ == "__main__":
    rng = np.random.default_rng(0)
    ins = dict(
        features=rng.integers(0, V, (8192, Fdim)).astype(np.int32),
        feature_values=rng.random((8192, Fdim), np.float32),
        emb_table=(rng.standard_normal((V, D)) * 0.01).astype(np.float32),
        bias_table=np.zeros((V, 1), np.float32),
        global_bias=np.zeros((1,), np.float32),
        att_w1=(rng.standard_normal((A, D)) / 8).astype(np.float32),
        att_b1=np.zeros((A,), np.float32),
        att_w2=rng.standard_normal((A,)).astype(np.float32),
        pred_w=np.ones((D,), np.float32),
    )
    out = kernel(**ins)
    print("out", out.shape, out[:8])
